# revision 14
# baseline (speedup 1.0000x reference)
"""Trainium2 Bass kernel for the AttentionBlock problem — v2.

Full inputs:  x [16, 64, 64, 64] f32, w_theta [8, 64], w_phi [8, 64],
              w_g [32, 64], w_o [64, 32], gamma [] (all f32).
Sharding: data-parallel over batch, 2 samples per core on 8 NeuronCores.

v2 redesign vs the 80143ns baseline (ACT exp, 71.4us busy, was the
bottleneck there):

1. theta folded into pooled phi: scoresT[t,s] = sum_k phiT[k,t] x[k,s]
   with phiT = (A*w_theta)^T @ phi — no theta conv and no theta copies;
   scores contract directly against x held in SBUF.
2. x is partition-stacked [128, 2048] per sample (each 1024-s chunk's two
   512-halves on partitions 0:64 / 64:128) so convs, scores, pools, adds
   and out-DMAs all use the full 128-partition width. All regular matmuls
   keep tile_position (0,0) — operand half-selection comes from
   zero-padded weight blocks (mixing PE quadrant configs, and any matmul
   dst at partition base 64, breaks on hardware). The conv pair is ONE
   matmul via the block-diagonal [wct|0; 0|wct] lhsT.
3. exp offload: 3 of 8 tiles per chunk (slots 0, 4, 6) run on DVE via the
   Schraudolph bit trick: scores arrive pre-scaled by A=128/ln2 (folded
   into w_theta), one tensor_scalar (add 16250, max 0) -> int16 rounds,
   and those int16 bits ARE the bf16 exp approximation. ACT tiles use
   exact exp with scale=1/A. End-to-end rel err ~1.9e-3 (gate 2e-2).
4. attn computed transposed: [128 s, 65] = [oU^T | Z] psum groups of four
   s-tiles per bank (one start / one stop per bank — PSUM start marks the
   whole 2KB bank pending-zero), halving attn PE columns vs [c, s] form;
   recip on a [128, 4] strided Z view, normalize is ONE broadcast
   tensor_tensor per group, PE transposes the bf16 result back (transposes
   tolerate partition-offset dst) into a psum bank, and a single
   [128, 512] DVE add applies the residual for a whole chunk.
5. schedule: slot t emits exp(t) (reading the scores tile emitted last
   slot), deferred phase-A units, scores(t+1), then prev-chunk attn, so
   scores PE work stays ahead of ACT; phase-A(0) units interleave into
   body 0; the weight/identity DMAs ride behind the first x block; the
   fine body splits its last exp and finishes per 64-partition half.

6. scores triple-buffering: the scores PSUM ring ran with 3 x [128,1024]
   buffers (6 banks) by squeezing the at pool to ONE bank (the two attn
   groups of a body never overlap in time; unit gw tiles pop at slots
   3/7 right after a group's norm frees the bank; the fine body's second
   own-group borrows a scores-ring tile) and the mix pool to ONE bank.
   This removes the sc(t)-waits-exp(t-2) ring stall entirely (~3us), at
   the cost of unit matmuls stalling briefly on the 1-buf mix ring —
   absorbed by the deeper scores ring. Per-body DVE exp slots are tuned
   per body (_DVEMAP); ~20 of 64 tiles run on DVE.

7. fine tail: per-group chains (exp half -> attn tt7 -> norm ->
   transposes) run with separate tp tiles (g0 from the mix ring, g1 from
   the freed at-ring slot), and BOTH residual adds + dual-queue DMAs are
   emitted last, so group 1's norm is never queued behind group 0's add
   on the in-order DVE engine.

Engine busies ~48-51us each (balanced). Measured: 70627 ns (TimelineSim,
= harness metric), rel err 1.85e-3 on device; baseline was 80143 ns.
"""

import os
import sys

if "/opt/trn_rl_repo" not in sys.path:
    sys.path.insert(0, "/opt/trn_rl_repo")

_STAGE = int(os.environ.get("K2_STAGE", "99"))  # debug: truncate bodies
_UNITS = int(os.environ.get("K2_UNITS", "99"))  # debug: truncate units
_UPARTS = os.environ.get("K2_UPARTS", "cprg")   # debug: unit pieces

import ml_dtypes
import numpy as np

import concourse.bass as bass
import concourse.tile as tile
from concourse import bacc, mybir
from concourse.bass_utils import run_bass_kernel_spmd

F32 = mybir.dt.float32
F32R = mybir.dt.float32r
BF16 = mybir.dt.bfloat16
I16 = mybir.dt.int16
AF = mybir.ActivationFunctionType
ALU = mybir.AluOpType

B, C, H, W = 16, 64, 64, 64
S = H * W            # 4096
T = S // 4           # 1024
NCORES = 8
BLOC = B // NCORES   # 2 samples per core
NT = 8               # t-tiles per sample
CHUNK = 1024         # s-chunk
NCH = S // CHUNK     # 4 chunks per sample
HEAD = 128           # conv-weight head columns in front of x
A = 128.0 / float(np.log(2.0))
RA = 1.0 / A
BOFFC = 127.0 * 128.0 - 6.0   # schraudolph bias (c=-6 centering)

_OUT = [None]
_XW0 = [None]
_IDENT = [None]
_WB = [None]       # bf16 weight blocks (phiT lhsT variants + wog variants)
_HEAD = [None]

# prev-chunk attn schedule: slot -> (group, [t-tiles]); group recip+mul
# emitted right after its last mms (slots 3 and 6)
_PREV_SCHED = {1: (0, [0, 1, 2]), 2: (0, [3, 4, 5]), 3: (0, [6, 7]),
               4: (1, [0, 1, 2]), 5: (1, [3, 4, 5]), 6: (1, [6, 7])}
# fine body: prev attn moved earlier so the at-ring frees in time for the
# fine body's own groups (allocated at slots 4 and 5)
_PREV_SCHED_F = {0: (0, [0, 1, 2, 3]), 1: (0, [4, 5, 6, 7]),
                 2: (1, [0, 1, 2, 3]), 3: (1, [4, 5, 6, 7])}
# fine body's own attn: slot -> [(group, [t-tiles])]; a tile tt only
# appears at slot >= tt+1 so PE never waits on ACT; tt7 handled in the
# split tail
_FINE_OWN = {4: [(0, [0, 1, 2])], 5: [(0, [3, 4]), (1, [0, 1, 2])],
             6: [(0, [5, 6]), (1, [3, 4])],
             7: [(1, [5, 6])]}
# exp-on-DVE slots per body index (bodies in emission order)
_USLOTS = tuple(int(c) for c in os.environ.get("K2_US", "135"))
_ULATE = os.environ.get("K2_UL", "e")
_DVEMAP = os.environ.get(
    "K2_DVEMAP", "6,046,046,14,04,046,046,046").split(",")
_DVE_SLOTS = {i: tuple(int(c) for c in _DVEMAP[i]) for i in range(8)}


def _phase_a(nc, pools, s, x_ext, mid=None):
    """Load x, fused conv pairs, merged pools, phiT and gw matmuls."""
    (pp_sc, pp_at, pp_mix, p_samp, p_chunk) = pools

    xw = p_samp.tile([128, HEAD + 2048], F32R, tag="x_sb")
    if s == 0:
        _XW0[0] = xw
        qs = [(0, HEAD + 512), (HEAD + 512, HEAD + 1024),
              (HEAD + 1024, HEAD + 1536), (HEAD + 1536, HEAD + 2048)]
    else:
        qs = [(HEAD, HEAD + 1024), (HEAD + 1024, HEAD + 2048)]
    for i, (lo, hi) in enumerate(qs):
        if i == 0 and s == 0 and os.environ.get("K2_FSPLIT", "0") == "1":
            # first block split across two DMA queues: halves transfer in
            # parallel so the first conv starts earlier
            mid2 = (lo + hi) // 2
            nc.scalar.dma_start(xw[:, lo:mid2], x_ext[s, :, lo:mid2])
            nc.sync.dma_start(xw[:, mid2:hi], x_ext[s, :, mid2:hi])
        else:
            nc.sync.dma_start(xw[:, lo:hi], x_ext[s, :, lo:hi])
        if i == 0 and mid is not None:
            mid()

    pg = p_samp.tile([128, 512], BF16, tag="pg")
    phiT = p_samp.tile([128, 2048], F32R, tag="phiT")
    gwo = p_samp.tile([128, NT * 65], BF16, tag="gwo")
    ones_v = gwo[:].rearrange("p (t c) -> p t c", c=65)[:, :, 64]
    nc.vector.memset(ones_v, 1.0)

    def unit(q):
        # All matmuls use full-128 contractions at tile_position (0, 0):
        # mixing PE quadrant configs (base-64 operand slices) breaks on hw,
        # so half-selection comes from zero-padded weight blocks instead.
        up = _UPARTS
        xw0 = _XW0[0]
        wb = _WB[0]
        if "c" not in up:
            return
        # conv pair in ONE matmul: lhsT rows 0:64 = [wct|0] (x-half 0 ->
        # out rows 0:64), rows 64:128 = [0|wct] (x-half 1 -> rows 64:128)
        ps_cv = pp_mix.tile([128, 512], F32, tag="mix", name=f"cv_{s}_{q}")
        nc.tensor.matmul(
            ps_cv[:],
            xw0[:, 0:128],
            xw[:, HEAD + 512 * q:HEAD + 512 * (q + 1)],
            start=True, stop=True,
        )
        if "p" not in up:
            return
        # merged 2x2 maxpool of both chunks in ONE tensor_reduce
        sv = ps_cv[:].rearrange("p (a hh wo ww) -> p a wo hh ww",
                                a=4, hh=2, wo=32, ww=2)
        dv = pg[:, 128 * q:128 * (q + 1)].rearrange("p (a wo) -> p a wo", wo=32)
        nc.vector.tensor_reduce(dv, sv, mybir.AxisListType.XY, ALU.max)
        if "r" not in up:
            return
        # phiT variants: tile t=2q+h, scores-half v; lhsT = wthA zero-padded
        # to select phi rows (0:8 even / 64:72 odd) and k-half columns
        ps_ph = pp_mix.tile([128, 512], F32, tag="mix", name=f"ph_{s}_{q}")
        for h in range(2):
            for v in range(2):
                nc.tensor.matmul(
                    ps_ph[:, 256 * h + 128 * v:256 * h + 128 * v + 128],
                    wb[:, 128 * (2 * h + v):128 * (2 * h + v) + 128],
                    pg[:, 128 * q:128 * (q + 1)],
                    start=True, stop=True,
                )
        nc.vector.tensor_copy(phiT[:, 512 * q:512 * (q + 1)],
                              ps_ph[:].bitcast(F32R))
        if "g" not in up:
            return
        # gw blocks: rhs = wog zero-padded to rows 32:64 (even) / 96:128
        # (odd); lhsT = full pg block
        ps_gw = pp_at.tile([128, 260], F32, tag="at", name=f"gw_{s}_{q}")
        for h in range(2):
            nc.tensor.matmul(
                ps_gw[:, 65 * h:65 * h + 64],
                pg[:, 128 * q:128 * (q + 1)],
                wb[:, 512 + 64 * h:576 + 64 * h],
                start=True, stop=True,
            )
        nc.vector.tensor_copy(
            gwo[:, 130 * q:130 * q + 130].rearrange(
                "p (h c) -> p h c", c=65)[:, :, 0:64],
            ps_gw[:, 0:130].rearrange("p (h c) -> p h c", c=65)[:, :, 0:64],
        )

    handles = (xw, pg, phiT, gwo)
    units = [lambda q=q: unit(q) for q in range(min(4, _UNITS))]
    return handles, units


def _act_reciprocal(nc, out, in_):
    """Reciprocal on ACT (table-based, ~1e-3 accurate — fine for the 2e-2
    gate; bass's wrapper refuses it, so emit the raw instruction)."""
    inputs = [nc.scalar.lower_ap(in_)]
    for v in (0.0, 1.0, 0.0):  # bias, scale, alpha immediates
        inputs.append(mybir.ImmediateValue(dtype=mybir.dt.float32, value=v))
    return nc.scalar.add_instruction(
        mybir.InstActivation(
            name=nc.scalar.bass.get_next_instruction_name(),
            func=AF.Reciprocal,
            ins=inputs,
            outs=[nc.scalar.lower_ap(out)],
        )
    )


def _emit_group_norm(nc, pools, at_g, oT_g, rz, on_act=False):
    """recip of the 4 Z columns + normalize -> oT bf16.

    on_act: run on the ACT engine (idle in the fine tail) — table recip +
    four Copy-with-scale-AP multiplies — so the chain doesn't queue behind
    DVE's other tail work.
    """
    zin = at_g[:, 0:260].rearrange("p (j c) -> p j c", c=65)
    if on_act:
        _act_reciprocal(nc, rz[:], zin[:, :, 64])
        for j in range(4):
            nc.scalar.activation(
                oT_g[:, 64 * j:64 * j + 64],
                at_g[:, 65 * j:65 * j + 64],
                AF.Copy, scale=rz[:, j:j + 1],
            )
        return
    nc.vector.reciprocal(rz[:], zin[:, :, 64])
    nc.vector.tensor_tensor(
        oT_g[:],
        zin[:, :, 0:64],
        rz[:].rearrange("p (j o) -> p j o", o=1).broadcast_to([128, 4, 64]),
        ALU.mult,
    )


def _emit_finish(nc, pools, s, ch, handles, oT, tp_name, tp_box=None,
                 only_g=None):
    """transposes into the tp bank + residual add + store.

    With only_g, finishes just that group's half (rows 64g:64g+64): used to
    split the fine tail so group 0's chain overlaps group 1's exp/attn.
    tp_box carries the tp tile across the two half-calls.
    """
    (pp_sc, pp_at, pp_mix, p_samp, p_chunk) = pools
    xw = handles[s][0]
    if tp_box is None:
        tp_box = [None]
    if tp_box[0] is None:
        tp_box[0] = pp_mix.tile([128, 512], BF16, tag="mix", name=tp_name)
    tp = tp_box[0]
    gs = (0, 1) if only_g is None else (only_g,)
    for g in gs:
        for j in range(4):
            nc.tensor.matmul(
                tp[64 * g:64 * g + 64, 128 * j:128 * j + 128],
                oT[g][:, 64 * j:64 * j + 64],
                _IDENT[0][:],
                is_transpose=True, start=True, stop=True,
            )
    s0 = CHUNK * ch
    if only_g is None:
        out_sb = p_chunk.tile([128, 512], F32, tag="out_sb",
                              name=f"out_{s}_{ch}", bufs=3)
        nc.vector.tensor_tensor(
            out_sb[:], tp[:],
            xw[:, HEAD + 512 * ch:HEAD + 512 * (ch + 1)].bitcast(F32),
            ALU.add,
        )
        nc.sync.dma_start(_OUT[0][s, :, s0:s0 + 512], out_sb[0:64, :])
        nc.sync.dma_start(_OUT[0][s, :, s0 + 512:s0 + 1024], out_sb[64:128, :])
    else:
        g = only_g
        out_sb = p_chunk.tile([128, 512], F32, tag="out_sb",
                              name=f"out_{s}_{ch}_{g}", bufs=3)
        nc.vector.tensor_tensor(
            out_sb[64 * g:64 * g + 64, :], tp[64 * g:64 * g + 64, :],
            xw[64 * g:64 * g + 64,
               HEAD + 512 * ch:HEAD + 512 * (ch + 1)].bitcast(F32),
            ALU.add,
        )
        eng = nc.scalar if (g == 1 and "d" in os.environ.get("K2_TAIL", "")) \
            else nc.sync
        eng.dma_start(_OUT[0][s, :, s0 + 512 * g:s0 + 512 * (g + 1)],
                      out_sb[64 * g:64 * g + 64, :])


def _emit_chunk(nc, pools, s, ch, handles, prev, body_idx, fine=False,
                units=(), nxt=None):
    """Slot t: exp(t) [reading the scores tile emitted last slot], then
    deferred phase-A units, then scores(t+1) — so the scores PE work for
    the next exp is always queued ahead of attn work and ACT never
    bubbles — then prev-chunk attn (+ fine-own attn)."""
    (pp_sc, pp_at, pp_mix, p_samp, p_chunk) = pools
    units = list(units)
    xw, pg, phiT, gwo = handles[s]

    expT = p_chunk.tile([128, NT * CHUNK], BF16, tag="expT",
                        name=f"expT_{s}_{ch}",
                        bufs=int(os.environ.get("K2_EB", "4")))
    dve_slots = _DVE_SLOTS.get(body_idx, ())

    at_prev = [None, None]
    oT_prev = [None, None]
    if prev is not None:
        ps_, ch_, expT_ = prev
        gwo_ = handles[ps_][3]
    at_own = [None, None]
    oT_own = [None, None]
    tp_prev = [None]
    tp_own = [None]

    # PSUM start marks the whole 2KB bank pending-zero (first write per
    # address then overwrites), so emit exactly ONE start on the first mm
    # into each at tile and ONE stop on the last of its 32; the 4 j-ranges
    # interleave freely in between.
    at_count = {}

    def attn_mm(at_g, e_, g_, g, tts, key):
        n = at_count.get(key, 0)
        for tt in tts:
            for j in range(4):
                nc.tensor.matmul(
                    at_g[:, 65 * j:65 * j + 65],
                    e_[:, CHUNK * tt + 128 * (4 * g + j):
                       CHUNK * tt + 128 * (4 * g + j) + 128],
                    g_[:, 65 * tt:65 * tt + 65],
                    start=(n == 0), stop=(n == 31),
                )
                n += 1
        at_count[key] = n

    def sc_mm(ps_sc, t, hh, s2=None, ch2=None):
        xw2, pg2, phiT2, gwo2 = handles[s2 if s2 is not None else s]
        c2 = ch2 if ch2 is not None else ch
        off = 512 * (t // 2) + 256 * (t % 2) + 128 * hh
        nc.tensor.matmul(
            ps_sc[:, 512 * hh:512 * hh + 512],
            phiT2[:, off:off + 128],
            xw2[:, HEAD + 512 * c2:HEAD + 512 * (c2 + 1)],
            start=True, stop=True,
        )

    def norm_group(kind, at_g, oT_list, g, sc, cc, on_act=False):
        oT_list[g] = p_chunk.tile([128, 256], BF16, tag=f"oT{g}",
                                  name=f"oT_{kind}_{sc}_{cc}_{g}", bufs=2)
        rz = p_chunk.tile([128, 4], F32, tag=f"rz{g}",
                          name=f"rz_{kind}_{sc}_{cc}_{g}", bufs=2)
        _emit_group_norm(nc, pools, at_g, oT_list[g], rz, on_act=on_act)

    prev_sched = _PREV_SCHED_F if fine else _PREV_SCHED
    pre_t0, _HEAD[0] = _HEAD[0], None
    cur_sc = pre_t0

    for t in range(NT):
        # ---- exp(t) from cur_sc (emitted at slot t-1 / pre-emitted)
        if s == 0 and ch == 0 and t == 0:
            cur_sc = pp_sc.tile([128, CHUNK], F32, tag="sc",
                                name=f"sc_{s}_{ch}_0")
            for hh in range(2):
                sc_mm(cur_sc, 0, hh)
                nc.scalar.activation(
                    expT[:, 512 * hh:512 * hh + 512],
                    cur_sc[:, 512 * hh:512 * hh + 512], AF.Exp, scale=RA,
                )
        elif fine and t == NT - 1:
            # interleaved split tail: emit group g's last-tile exp half and
            # then that group's whole chain (attn tt7 + norm + transposes +
            # add + DMA) BEFORE the other half's exp — the chain then can't
            # pick up a false dependency on the later exp half, and each
            # group gets its own tp tile so add(g0) doesn't serialize the
            # g1 transposes through tile write-after-read ordering
            for g_, tts_ in _FINE_OWN.get(t, ()):
                attn_mm(at_own[g_], expT, gwo, g_, tts_, ("o", g_))
            # per-group: exp half -> tt7 mms -> norm -> transposes (own tp
            # tile); the residual adds + DMAs go LAST so g1's norm is never
            # stuck behind g0's add in the in-order DVE queue
            tps = []
            for g_ in range(2):
                nc.scalar.activation(
                    expT[:, CHUNK * t + 512 * g_:CHUNK * t + 512 * (g_ + 1)],
                    cur_sc[:, 512 * g_:512 * (g_ + 1)], AF.Exp, scale=RA,
                )
                attn_mm(at_own[g_], expT, gwo, g_, [7], ("o", g_))
                norm_group("o", at_own[g_], oT_own, g_, s, ch)
                # g0's tp from the mix ring; g1's from the at-ring slot
                # that own-group 0's norm just freed — a single-buf ring
                # cannot hold both tps at once without deadlocking on the
                # deferred adds
                pool_g = pp_mix if g_ == 0 else pp_at
                tp_g = pool_g.tile([128, 512], BF16,
                                   tag=("mix" if g_ == 0 else "at"),
                                   name=f"tp_{s}_{ch}_{g_}")
                for j in range(4):
                    nc.tensor.matmul(
                        tp_g[64 * g_:64 * g_ + 64, 128 * j:128 * j + 128],
                        oT_own[g_][:, 64 * j:64 * j + 64],
                        _IDENT[0][:],
                        is_transpose=True, start=True, stop=True,
                    )
                tps.append(tp_g)
            s0 = CHUNK * ch
            for g_ in (0, 1):
                out_sb = p_chunk.tile([128, 512], F32, tag="out_sb",
                                      name=f"out_{s}_{ch}_{g_}", bufs=3)
                nc.vector.tensor_tensor(
                    out_sb[64 * g_:64 * g_ + 64, :],
                    tps[g_][64 * g_:64 * g_ + 64, :],
                    xw[64 * g_:64 * g_ + 64,
                       HEAD + 512 * ch:HEAD + 512 * (ch + 1)].bitcast(F32),
                    ALU.add,
                )
                (nc.scalar if g_ == 1 else nc.sync).dma_start(
                    _OUT[0][s, :, s0 + 512 * g_:s0 + 512 * (g_ + 1)],
                    out_sb[64 * g_:64 * g_ + 64, :])
            continue
        elif t in dve_slots:
            # split on the psum bank boundary: the successor scores matmul
            # into this tile's first bank can start after the first half
            if os.environ.get("K2_DSPLIT", "0") == "1":
                for hh in range(2):
                    nc.vector.tensor_scalar(
                        expT[:, CHUNK * t + 512 * hh:
                             CHUNK * t + 512 * (hh + 1)].bitcast(I16),
                        cur_sc[:, 512 * hh:512 * (hh + 1)],
                        BOFFC, 0.0, ALU.add, ALU.max,
                    )
            else:
                nc.vector.tensor_scalar(
                    expT[:, CHUNK * t:CHUNK * (t + 1)].bitcast(I16),
                    cur_sc[:], BOFFC, 0.0, ALU.add, ALU.max,
                )
        else:
            nc.scalar.activation(
                expT[:, CHUNK * t:CHUNK * (t + 1)], cur_sc[:], AF.Exp,
                scale=RA,
            )

        # ---- body 0 only: units must precede the scores that read their
        # phiT (in-order PE would deadlock otherwise); elsewhere units are
        # emitted at slot end so they never delay the scores chain
        if body_idx == 0 and t in (1, 3, 5) and units:
            units.pop(0)()

        # ---- scores for t+1 (or next body's tile 0)
        if t < NT - 1:
            cur_sc = pp_sc.tile([128, CHUNK], F32, tag="sc",
                                name=f"sc_{s}_{ch}_{t + 1}")
            sc_mm(cur_sc, t + 1, 0)
            sc_mm(cur_sc, t + 1, 1)
        elif nxt is not None:
            s2, ch2 = nxt
            ps_head = pp_sc.tile([128, CHUNK], F32, tag="sc",
                                 name=f"sc_{s2}_{ch2}_0")
            sc_mm(ps_head, 0, 0, s2, ch2)
            sc_mm(ps_head, 0, 1, s2, ch2)
            _HEAD[0] = ps_head

        # ---- prev-chunk attn
        if prev is not None and t in prev_sched:
            g, tts = prev_sched[t]
            if at_prev[g] is None:
                at_prev[g] = pp_at.tile([128, 260], F32, tag="at",
                                        name=f"at_{ps_}_{ch_}_{g}")
            attn_mm(at_prev[g], expT_, gwo_, g, tts, ("p", g))
            if tts[-1] == 7:
                norm_group("p", at_prev[g], oT_prev, g, ps_, ch_)
                if fine and g == 1:
                    _emit_finish(nc, pools, ps_, ch_, handles, oT_prev,
                                 f"tp_{ps_}_{ch_}", tp_prev)

        # ---- fine body: own attn
        if fine and t in _FINE_OWN:
            for g, tts in _FINE_OWN[t]:
                if at_own[g] is None:
                    if g == 0:
                        at_own[g] = pp_at.tile([128, 260], F32, tag="at",
                                               name=f"at_{s}_{ch}_{g}")
                    else:
                        # the 1-buf at-ring still holds own group 0; borrow
                        # a scores-ring tile (2 banks, first 260 cols used)
                        at_own[g] = pp_sc.tile([128, CHUNK], F32, tag="sc",
                                               name=f"at_{s}_{ch}_{g}")[:, 0:260]
                attn_mm(at_own[g], expT, gwo, g, tts, ("o", g))

        # ---- deferred phase-A units for the next sample: slot end, and
        # only at slots just after an at-group norm (3) or after both
        # groups are done (7) — the 1-buf at-ring frees exactly there
        if body_idx != 0 and t in (3, 7) and units:
            units.pop(0)()

    if fine:
        return None

    if prev is not None:
        _emit_finish(nc, pools, ps_, ch_, handles, oT_prev, f"tp_{ps_}_{ch_}",
                     tp_prev)
    return (s, ch, expT)


def build_nc():
    nc = bacc.Bacc("TRN2", target_bir_lowering=False, debug=False,
                   num_devices=NCORES)
    x_ext = nc.dram_tensor("x", [BLOC, 128, HEAD + 2048], F32R,
                           kind="ExternalInput").ap()
    ident_ext = nc.dram_tensor("ident", [128, 128], BF16,
                               kind="ExternalInput").ap()
    wb_ext = nc.dram_tensor("wb", [128, 640], BF16,
                            kind="ExternalInput").ap()
    out_ext = nc.dram_tensor("out", [BLOC, C, S], F32,
                             kind="ExternalOutput").ap()

    with tile.TileContext(nc) as tc:
        with (
            tc.tile_pool(name="wpool", bufs=1) as p_w,
            tc.tile_pool(name="samp", bufs=2) as p_samp,
            tc.tile_pool(name="chunk", bufs=2) as p_chunk,
            tc.tile_pool(name="ppsc", bufs=3, space="PSUM") as pp_sc,
            tc.tile_pool(name="ppat", bufs=1, space="PSUM") as pp_at,
            tc.tile_pool(name="ppmix", bufs=1, space="PSUM") as pp_mix,
        ):
            ident_sb = p_w.tile([128, 128], BF16, tag="ident")
            wb_sb = p_w.tile([128, 640], BF16, tag="wb")
            _IDENT[0] = ident_sb
            _WB[0] = wb_sb
            # dummy exp: hoist the ACT exp-table load off the critical
            # path (tiny memset so the table load starts immediately)
            dummy_sb = p_w.tile([1, 2], F32, tag="dummy_sb")
            nc.vector.memset(dummy_sb[:], 0)
            nc.scalar.activation(dummy_sb[0:1, 0:2], dummy_sb[0:1, 0:2],
                                 AF.Exp)
            _OUT[0] = out_ext
            pools = (pp_sc, pp_at, pp_mix, p_samp, p_chunk)
            handles = [None] * BLOC

            def mid_dmas():
                # wb is needed by unit 0 (~2.5us in); ident only by the
                # first finish (~12us); both ride after the first x block
                nc.sync.dma_start(wb_sb[:], wb_ext[:])
                nc.sync.dma_start(ident_sb[:], ident_ext[:])

            handles[0], units0 = _phase_a(nc, pools, 0, x_ext, mid=mid_dmas)
            units0[0]()
            prev = None
            pending = units0[1:]
            seq = [(s, ch) for s in range(BLOC) for ch in range(NCH)]
            seq = seq[:_STAGE]
            for i, (s, ch) in enumerate(seq):
                last = i == len(seq) - 1 and _STAGE >= 8
                nxt = None if i == len(seq) - 1 else seq[i + 1]
                take = 3 if i == 0 else 2
                prev = _emit_chunk(nc, pools, s, ch, handles, prev,
                                   body_idx=i, fine=last,
                                   units=pending[:take], nxt=nxt)
                pending = pending[take:]
                if ch == 0 and s + 1 < BLOC:
                    handles[s + 1], pending = _phase_a(nc, pools, s + 1,
                                                       x_ext)

    nc.compile()
    return nc


_NC_CACHE = None


def _get_nc():
    global _NC_CACHE
    if _NC_CACHE is None:
        _NC_CACHE = build_nc()
    return _NC_CACHE


def _pack_inputs(x, w_theta, w_phi, w_g, w_o, gamma_f):
    """Host-side packing: conv head + bf16 weight blocks + stacked x."""
    head = np.zeros((128, HEAD), dtype=np.float32)
    wct = np.zeros((64, 64), dtype=np.float32)
    wct[:, 0:8] = w_phi.T
    wct[:, 32:64] = w_g.T
    # conv lhsT: rows 0:64 = [wct|0], rows 64:128 = [0|wct]
    head[0:64, 0:64] = wct
    head[64:128, 64:128] = wct

    wb = np.zeros((128, 640), dtype=ml_dtypes.bfloat16)
    wthA = (A * w_theta).astype(ml_dtypes.bfloat16)      # [8, 64]
    wog = ((gamma_f * w_o).T).astype(ml_dtypes.bfloat16)  # [32, 64]
    # phiT lhsT variants T(2h+v) at cols 128*(2h+v):
    #   h: phi row block (0:8 even tile / 64:72 odd), v: k-half column
    for h in range(2):
        for v in range(2):
            c0 = 128 * (2 * h + v)
            wb[64 * h:64 * h + 8, c0 + 64 * v:c0 + 64 * v + 64] = wthA
    # wog variants: rows 32:64 (even) / 96:128 (odd)
    wb[32:64, 512:576] = wog
    wb[96:128, 576:640] = wog

    xr = x.reshape(B, 64, 4, 2, 512)
    x128 = np.ascontiguousarray(xr.transpose(0, 3, 1, 2, 4)).reshape(
        B, 128, 2048)
    xcat = np.zeros((B, 128, HEAD + 2048), dtype=np.float32)
    xcat[:, :, HEAD:] = x128
    xcat[0::BLOC, :, 0:HEAD] = head[None, :, :]
    ident = np.eye(128, dtype=ml_dtypes.bfloat16)
    return xcat, ident, wb


def kernel(x, w_theta, w_phi, w_g, w_o, gamma):
    x = np.ascontiguousarray(np.asarray(x, dtype=np.float32))
    w_theta = np.asarray(w_theta, dtype=np.float32)
    w_phi = np.asarray(w_phi, dtype=np.float32)
    w_g = np.asarray(w_g, dtype=np.float32)
    w_o = np.asarray(w_o, dtype=np.float32)
    gamma_f = float(np.asarray(gamma, dtype=np.float32))

    xcat, ident, wb = _pack_inputs(x, w_theta, w_phi, w_g, w_o, gamma_f)

    nc = _get_nc()
    in_maps = [
        {
            "x": np.ascontiguousarray(xcat[i * BLOC:(i + 1) * BLOC]),
            "ident": ident,
            "wb": wb,
        }
        for i in range(NCORES)
    ]
    res = run_bass_kernel_spmd(nc, in_maps, core_ids=list(range(NCORES)))
    out = np.concatenate([res.results[i]["out"] for i in range(NCORES)],
                         axis=0)
    return out.reshape(B, C, H, W).astype(np.float32)


if __name__ == "__main__":
    rng = np.random.default_rng(0)
    ins = {
        "x": rng.standard_normal((B, C, H, W), dtype=np.float32),
        "w_theta": (rng.standard_normal((8, 64)) / 8.0).astype(np.float32),
        "w_phi": (rng.standard_normal((8, 64)) / 8.0).astype(np.float32),
        "w_g": (rng.standard_normal((32, 64)) / np.sqrt(64)).astype(np.float32),
        "w_o": (rng.standard_normal((64, 32)) / np.sqrt(32)).astype(np.float32),
        "gamma": np.float32(0.7),
    }
    out = kernel(**ins)
    print("out", out.shape, out.dtype, np.abs(out).mean())


# revision 15
# speedup vs baseline: 1.0184x; 1.0184x over previous
"""Trainium2 Bass kernel for the AttentionBlock problem — v2.

Full inputs:  x [16, 64, 64, 64] f32, w_theta [8, 64], w_phi [8, 64],
              w_g [32, 64], w_o [64, 32], gamma [] (all f32).
Sharding: data-parallel over batch, 2 samples per core on 8 NeuronCores.

v2 redesign vs the 80143ns baseline (ACT exp, 71.4us busy, was the
bottleneck there):

1. theta folded into pooled phi: scoresT[t,s] = sum_k phiT[k,t] x[k,s]
   with phiT = (A*w_theta)^T @ phi — no theta conv and no theta copies;
   scores contract directly against x held in SBUF.
2. x is partition-stacked [128, 2048] per sample (each 1024-s chunk's two
   512-halves on partitions 0:64 / 64:128) so convs, scores, pools, adds
   and out-DMAs all use the full 128-partition width. All regular matmuls
   keep tile_position (0,0) — operand half-selection comes from
   zero-padded weight blocks (mixing PE quadrant configs, and any matmul
   dst at partition base 64, breaks on hardware). The conv pair is ONE
   matmul via the block-diagonal [wct|0; 0|wct] lhsT.
3. exp offload: 3 of 8 tiles per chunk (slots 0, 4, 6) run on DVE via the
   Schraudolph bit trick: scores arrive pre-scaled by A=128/ln2 (folded
   into w_theta), one tensor_scalar (add 16250, max 0) -> int16 rounds,
   and those int16 bits ARE the bf16 exp approximation. ACT tiles use
   exact exp with scale=1/A. End-to-end rel err ~1.9e-3 (gate 2e-2).
4. attn computed transposed: [128 s, 65] = [oU^T | Z] psum groups of four
   s-tiles per bank (one start / one stop per bank — PSUM start marks the
   whole 2KB bank pending-zero), halving attn PE columns vs [c, s] form;
   recip on a [128, 4] strided Z view, normalize is ONE broadcast
   tensor_tensor per group, PE transposes the bf16 result back (transposes
   tolerate partition-offset dst) into a psum bank, and a single
   [128, 512] DVE add applies the residual for a whole chunk.
5. schedule: slot t emits exp(t) (reading the scores tile emitted last
   slot), deferred phase-A units, scores(t+1), then prev-chunk attn, so
   scores PE work stays ahead of ACT; phase-A(0) units interleave into
   body 0; the weight/identity DMAs ride behind the first x block; the
   fine body splits its last exp and finishes per 64-partition half.

6. scores triple-buffering: the scores PSUM ring ran with 3 x [128,1024]
   buffers (6 banks) by squeezing the at pool to ONE bank (the two attn
   groups of a body never overlap in time; unit gw tiles pop at slots
   3/7 right after a group's norm frees the bank; the fine body's second
   own-group borrows a scores-ring tile) and the mix pool to ONE bank.
   This removes the sc(t)-waits-exp(t-2) ring stall entirely (~3us), at
   the cost of unit matmuls stalling briefly on the 1-buf mix ring —
   absorbed by the deeper scores ring. Per-body DVE exp slots are tuned
   per body (_DVEMAP); ~20 of 64 tiles run on DVE.

7. fine tail: per-group chains (exp half -> attn tt7 -> norm ->
   transposes) run with separate tp tiles (g0 from the mix ring, g1 from
   the freed at-ring slot), and BOTH residual adds + dual-queue DMAs are
   emitted last, so group 1's norm is never queued behind group 0's add
   on the in-order DVE engine.

Engine busies ~48-51us each (balanced). Measured: 70627 ns (TimelineSim,
= harness metric), rel err 1.85e-3 on device; baseline was 80143 ns.
"""

import os
import sys

if "/opt/trn_rl_repo" not in sys.path:
    sys.path.insert(0, "/opt/trn_rl_repo")

_STAGE = int(os.environ.get("K2_STAGE", "99"))  # debug: truncate bodies
_UNITS = int(os.environ.get("K2_UNITS", "99"))  # debug: truncate units
_UPARTS = os.environ.get("K2_UPARTS", "cprg")   # debug: unit pieces

import ml_dtypes
import numpy as np

import concourse.bass as bass
import concourse.tile as tile
from concourse import bacc, mybir
from concourse.bass_utils import run_bass_kernel_spmd

F32 = mybir.dt.float32
F32R = mybir.dt.float32r
BF16 = mybir.dt.bfloat16
I16 = mybir.dt.int16
AF = mybir.ActivationFunctionType
ALU = mybir.AluOpType

B, C, H, W = 16, 64, 64, 64
S = H * W            # 4096
T = S // 4           # 1024
NCORES = 8
BLOC = B // NCORES   # 2 samples per core
NT = 8               # t-tiles per sample
CHUNK = 1024         # s-chunk
NCH = S // CHUNK     # 4 chunks per sample
HEAD = 128           # conv-weight head columns in front of x
A = 128.0 / float(np.log(2.0))
RA = 1.0 / A
BOFFC = 127.0 * 128.0 - 6.0   # schraudolph bias (c=-6 centering)

_OUT = [None]
_XW0 = [None]
_IDENT = [None]
_WB = [None]       # bf16 weight blocks (phiT lhsT variants + wog variants)
_HEAD = [None]

# prev-chunk attn schedule: slot -> (group, [t-tiles]); group recip+mul
# emitted right after its last mms (slots 3 and 6)
_PREV_SCHED = {1: (0, [0, 1, 2]), 2: (0, [3, 4, 5]), 3: (0, [6, 7]),
               4: (1, [0, 1, 2]), 5: (1, [3, 4, 5]), 6: (1, [6, 7])}
# fine body: prev attn moved earlier so the at-ring frees in time for the
# fine body's own groups (allocated at slots 4 and 5)
_PREV_SCHED_F = {0: (0, [0, 1, 2, 3]), 1: (0, [4, 5, 6, 7]),
                 2: (1, [0, 1, 2, 3]), 3: (1, [4, 5, 6, 7])}
# fine body's own attn: slot -> [(group, [t-tiles])]; a tile tt only
# appears at slot >= tt+1 so PE never waits on ACT; tt7 handled in the
# split tail
_FINE_OWN = {4: [(0, [0, 1, 2])], 5: [(0, [3, 4]), (1, [0, 1, 2])],
             6: [(0, [5, 6]), (1, [3, 4])],
             7: [(1, [5, 6])]}
# exp-on-DVE slots per body index (bodies in emission order)
_USLOTS = tuple(int(c) for c in os.environ.get("K2_US", "135"))
_ULATE = os.environ.get("K2_UL", "e")
_DVEMAP = os.environ.get(
    "K2_DVEMAP", "6,046,046,14,04,046,046,046").split(",")
_DVE_SLOTS = {i: tuple(int(c) for c in _DVEMAP[i]) for i in range(8)}


def _phase_a(nc, pools, s, x_ext, mid=None):
    """Load x, fused conv pairs, merged pools, phiT and gw matmuls."""
    (pp_sc, pp_at, pp_mix, p_samp, p_chunk) = pools

    xw = p_samp.tile([128, HEAD + 2048], F32R, tag="x_sb")
    if s == 0:
        _XW0[0] = xw
        qs = [(0, HEAD + 512), (HEAD + 512, HEAD + 1024),
              (HEAD + 1024, HEAD + 1536), (HEAD + 1536, HEAD + 2048)]
    else:
        qs = [(HEAD, HEAD + 1024), (HEAD + 1024, HEAD + 2048)]
    for i, (lo, hi) in enumerate(qs):
        if i == 0 and s == 0 and os.environ.get("K2_FSPLIT", "0") == "1":
            # first block split across two DMA queues: halves transfer in
            # parallel so the first conv starts earlier
            mid2 = (lo + hi) // 2
            nc.scalar.dma_start(xw[:, lo:mid2], x_ext[s, :, lo:mid2])
            nc.sync.dma_start(xw[:, mid2:hi], x_ext[s, :, mid2:hi])
        else:
            nc.sync.dma_start(xw[:, lo:hi], x_ext[s, :, lo:hi])
        if i == 0 and mid is not None:
            mid()

    pg = p_samp.tile([128, 512], BF16, tag="pg")
    phiT = p_samp.tile([128, 2048], F32R, tag="phiT")
    gwo = p_samp.tile([128, NT * 65], BF16, tag="gwo")
    ones_v = gwo[:].rearrange("p (t c) -> p t c", c=65)[:, :, 64]
    nc.vector.memset(ones_v, 1.0)

    def unit(q):
        # All matmuls use full-128 contractions at tile_position (0, 0):
        # mixing PE quadrant configs (base-64 operand slices) breaks on hw,
        # so half-selection comes from zero-padded weight blocks instead.
        up = _UPARTS
        xw0 = _XW0[0]
        wb = _WB[0]
        if "c" not in up:
            return
        # conv pair in ONE matmul: lhsT rows 0:64 = [wct|0] (x-half 0 ->
        # out rows 0:64), rows 64:128 = [0|wct] (x-half 1 -> rows 64:128)
        ps_cv = pp_mix.tile([128, 512], F32, tag="mix", name=f"cv_{s}_{q}")
        nc.tensor.matmul(
            ps_cv[:],
            xw0[:, 0:128],
            xw[:, HEAD + 512 * q:HEAD + 512 * (q + 1)],
            start=True, stop=True,
        )
        if "p" not in up:
            return
        # merged 2x2 maxpool of both chunks in ONE tensor_reduce
        sv = ps_cv[:].rearrange("p (a hh wo ww) -> p a wo hh ww",
                                a=4, hh=2, wo=32, ww=2)
        dv = pg[:, 128 * q:128 * (q + 1)].rearrange("p (a wo) -> p a wo", wo=32)
        nc.vector.tensor_reduce(dv, sv, mybir.AxisListType.XY, ALU.max)
        if "r" not in up:
            return
        # phiT variants: tile t=2q+h, scores-half v; lhsT = wthA zero-padded
        # to select phi rows (0:8 even / 64:72 odd) and k-half columns
        ps_ph = pp_mix.tile([128, 512], F32, tag="mix", name=f"ph_{s}_{q}")
        for h in range(2):
            for v in range(2):
                nc.tensor.matmul(
                    ps_ph[:, 256 * h + 128 * v:256 * h + 128 * v + 128],
                    wb[:, 128 * (2 * h + v):128 * (2 * h + v) + 128],
                    pg[:, 128 * q:128 * (q + 1)],
                    start=True, stop=True,
                )
        nc.vector.tensor_copy(phiT[:, 512 * q:512 * (q + 1)],
                              ps_ph[:].bitcast(F32R))
        if "g" not in up:
            return
        # gw blocks: rhs = wog zero-padded to rows 32:64 (even) / 96:128
        # (odd); lhsT = full pg block
        ps_gw = pp_at.tile([128, 260], F32, tag="at", name=f"gw_{s}_{q}")
        for h in range(2):
            nc.tensor.matmul(
                ps_gw[:, 65 * h:65 * h + 64],
                pg[:, 128 * q:128 * (q + 1)],
                wb[:, 512 + 64 * h:576 + 64 * h],
                start=True, stop=True,
            )
        nc.vector.tensor_copy(
            gwo[:, 130 * q:130 * q + 130].rearrange(
                "p (h c) -> p h c", c=65)[:, :, 0:64],
            ps_gw[:, 0:130].rearrange("p (h c) -> p h c", c=65)[:, :, 0:64],
        )

    handles = (xw, pg, phiT, gwo)
    units = [lambda q=q: unit(q) for q in range(min(4, _UNITS))]
    return handles, units


def _act_reciprocal(nc, out, in_):
    """Reciprocal on ACT (table-based, ~1e-3 accurate — fine for the 2e-2
    gate; bass's wrapper refuses it, so emit the raw instruction)."""
    inputs = [nc.scalar.lower_ap(in_)]
    for v in (0.0, 1.0, 0.0):  # bias, scale, alpha immediates
        inputs.append(mybir.ImmediateValue(dtype=mybir.dt.float32, value=v))
    return nc.scalar.add_instruction(
        mybir.InstActivation(
            name=nc.scalar.bass.get_next_instruction_name(),
            func=AF.Reciprocal,
            ins=inputs,
            outs=[nc.scalar.lower_ap(out)],
        )
    )


def _emit_group_norm(nc, pools, at_g, oT_g, rz, on_act=False):
    """recip of the 4 Z columns + normalize -> oT bf16.

    on_act: run on the ACT engine (idle in the fine tail) — table recip +
    four Copy-with-scale-AP multiplies — so the chain doesn't queue behind
    DVE's other tail work.
    """
    zin = at_g[:, 0:260].rearrange("p (j c) -> p j c", c=65)
    if on_act:
        _act_reciprocal(nc, rz[:], zin[:, :, 64])
        for j in range(4):
            nc.scalar.activation(
                oT_g[:, 64 * j:64 * j + 64],
                at_g[:, 65 * j:65 * j + 64],
                AF.Copy, scale=rz[:, j:j + 1],
            )
        return
    nc.vector.reciprocal(rz[:], zin[:, :, 64])
    nc.vector.tensor_tensor(
        oT_g[:],
        zin[:, :, 0:64],
        rz[:].rearrange("p (j o) -> p j o", o=1).broadcast_to([128, 4, 64]),
        ALU.mult,
    )


def _emit_finish(nc, pools, s, ch, handles, oT, tp_name, tp_box=None,
                 only_g=None):
    """transposes into the tp bank + residual add + store.

    With only_g, finishes just that group's half (rows 64g:64g+64): used to
    split the fine tail so group 0's chain overlaps group 1's exp/attn.
    tp_box carries the tp tile across the two half-calls.
    """
    (pp_sc, pp_at, pp_mix, p_samp, p_chunk) = pools
    xw = handles[s][0]
    if tp_box is None:
        tp_box = [None]
    if tp_box[0] is None:
        tp_box[0] = pp_mix.tile([128, 512], BF16, tag="mix", name=tp_name)
    tp = tp_box[0]
    gs = (0, 1) if only_g is None else (only_g,)
    for g in gs:
        for j in range(4):
            nc.tensor.matmul(
                tp[64 * g:64 * g + 64, 128 * j:128 * j + 128],
                oT[g][:, 64 * j:64 * j + 64],
                _IDENT[0][:],
                is_transpose=True, start=True, stop=True,
            )
    s0 = CHUNK * ch
    if only_g is None:
        out_sb = p_chunk.tile([128, 512], F32, tag="out_sb",
                              name=f"out_{s}_{ch}", bufs=3)
        nc.vector.tensor_tensor(
            out_sb[:], tp[:],
            xw[:, HEAD + 512 * ch:HEAD + 512 * (ch + 1)].bitcast(F32),
            ALU.add,
        )
        nc.sync.dma_start(_OUT[0][s, :, s0:s0 + 512], out_sb[0:64, :])
        nc.sync.dma_start(_OUT[0][s, :, s0 + 512:s0 + 1024], out_sb[64:128, :])
    else:
        g = only_g
        out_sb = p_chunk.tile([128, 512], F32, tag="out_sb",
                              name=f"out_{s}_{ch}_{g}", bufs=3)
        nc.vector.tensor_tensor(
            out_sb[64 * g:64 * g + 64, :], tp[64 * g:64 * g + 64, :],
            xw[64 * g:64 * g + 64,
               HEAD + 512 * ch:HEAD + 512 * (ch + 1)].bitcast(F32),
            ALU.add,
        )
        eng = nc.scalar if (g == 1 and "d" in os.environ.get("K2_TAIL", "")) \
            else nc.sync
        eng.dma_start(_OUT[0][s, :, s0 + 512 * g:s0 + 512 * (g + 1)],
                      out_sb[64 * g:64 * g + 64, :])


def _emit_chunk(nc, pools, s, ch, handles, prev, body_idx, fine=False,
                units=(), nxt=None):
    """Slot t: exp(t) [reading the scores tile emitted last slot], then
    deferred phase-A units, then scores(t+1) — so the scores PE work for
    the next exp is always queued ahead of attn work and ACT never
    bubbles — then prev-chunk attn (+ fine-own attn)."""
    (pp_sc, pp_at, pp_mix, p_samp, p_chunk) = pools
    units = list(units)
    xw, pg, phiT, gwo = handles[s]

    expT = p_chunk.tile([128, NT * CHUNK], BF16, tag="expT",
                        name=f"expT_{s}_{ch}",
                        bufs=int(os.environ.get("K2_EB", "4")))
    dve_slots = _DVE_SLOTS.get(body_idx, ())

    at_prev = [None, None]
    oT_prev = [None, None]
    if prev is not None:
        ps_, ch_, expT_ = prev
        gwo_ = handles[ps_][3]
    at_own = [None, None]
    oT_own = [None, None]
    tp_prev = [None]
    tp_own = [None]

    # PSUM start marks the whole 2KB bank pending-zero (first write per
    # address then overwrites), so emit exactly ONE start on the first mm
    # into each at tile and ONE stop on the last of its 32; the 4 j-ranges
    # interleave freely in between.
    at_count = {}

    def attn_mm(at_g, e_, g_, g, tts, key):
        n = at_count.get(key, 0)
        for tt in tts:
            for j in range(4):
                nc.tensor.matmul(
                    at_g[:, 65 * j:65 * j + 65],
                    e_[:, CHUNK * tt + 128 * (4 * g + j):
                       CHUNK * tt + 128 * (4 * g + j) + 128],
                    g_[:, 65 * tt:65 * tt + 65],
                    start=(n == 0), stop=(n == 31),
                )
                n += 1
        at_count[key] = n

    def sc_mm(ps_sc, t, hh, s2=None, ch2=None):
        xw2, pg2, phiT2, gwo2 = handles[s2 if s2 is not None else s]
        c2 = ch2 if ch2 is not None else ch
        off = 512 * (t // 2) + 256 * (t % 2) + 128 * hh
        nc.tensor.matmul(
            ps_sc[:, 512 * hh:512 * hh + 512],
            phiT2[:, off:off + 128],
            xw2[:, HEAD + 512 * c2:HEAD + 512 * (c2 + 1)],
            start=True, stop=True,
        )

    def norm_group(kind, at_g, oT_list, g, sc, cc, on_act=False):
        oT_list[g] = p_chunk.tile([128, 256], BF16, tag=f"oT{g}",
                                  name=f"oT_{kind}_{sc}_{cc}_{g}", bufs=2)
        rz = p_chunk.tile([128, 4], F32, tag=f"rz{g}",
                          name=f"rz_{kind}_{sc}_{cc}_{g}", bufs=2)
        _emit_group_norm(nc, pools, at_g, oT_list[g], rz, on_act=on_act)

    prev_sched = _PREV_SCHED_F if fine else _PREV_SCHED
    pre_t0, _HEAD[0] = _HEAD[0], None
    cur_sc = pre_t0

    for t in range(NT):
        # ---- exp(t) from cur_sc (emitted at slot t-1 / pre-emitted)
        if s == 0 and ch == 0 and t == 0:
            cur_sc = pp_sc.tile([128, CHUNK], F32, tag="sc",
                                name=f"sc_{s}_{ch}_0")
            for hh in range(2):
                sc_mm(cur_sc, 0, hh)
                nc.scalar.activation(
                    expT[:, 512 * hh:512 * hh + 512],
                    cur_sc[:, 512 * hh:512 * hh + 512], AF.Exp, scale=RA,
                )
        elif fine and t == NT - 1:
            # interleaved split tail: emit group g's last-tile exp half and
            # then that group's whole chain (attn tt7 + norm + transposes +
            # add + DMA) BEFORE the other half's exp — the chain then can't
            # pick up a false dependency on the later exp half, and each
            # group gets its own tp tile so add(g0) doesn't serialize the
            # g1 transposes through tile write-after-read ordering
            for g_, tts_ in _FINE_OWN.get(t, ()):
                attn_mm(at_own[g_], expT, gwo, g_, tts_, ("o", g_))
            # per-group: exp half -> tt7 mms -> norm -> transposes (own tp
            # tile); the residual adds + DMAs go LAST so g1's norm is never
            # stuck behind g0's add in the in-order DVE queue
            tps = []
            for g_ in range(2):
                nc.scalar.activation(
                    expT[:, CHUNK * t + 512 * g_:CHUNK * t + 512 * (g_ + 1)],
                    cur_sc[:, 512 * g_:512 * (g_ + 1)], AF.Exp, scale=RA,
                )
                attn_mm(at_own[g_], expT, gwo, g_, [7], ("o", g_))
                norm_group("o", at_own[g_], oT_own, g_, s, ch)
                # g0's tp from the mix ring; g1's from the at-ring slot
                # that own-group 0's norm just freed — a single-buf ring
                # cannot hold both tps at once without deadlocking on the
                # deferred adds
                pool_g = pp_mix if g_ == 0 else pp_at
                tp_g = pool_g.tile([128, 512], BF16,
                                   tag=("mix" if g_ == 0 else "at"),
                                   name=f"tp_{s}_{ch}_{g_}")
                for j in range(4):
                    nc.tensor.matmul(
                        tp_g[64 * g_:64 * g_ + 64, 128 * j:128 * j + 128],
                        oT_own[g_][:, 64 * j:64 * j + 64],
                        _IDENT[0][:],
                        is_transpose=True, start=True, stop=True,
                    )
                tps.append(tp_g)
            s0 = CHUNK * ch
            for g_ in (0, 1):
                out_sb = p_chunk.tile([128, 512], F32, tag="out_sb",
                                      name=f"out_{s}_{ch}_{g_}", bufs=3)
                nc.vector.tensor_tensor(
                    out_sb[64 * g_:64 * g_ + 64, :],
                    tps[g_][64 * g_:64 * g_ + 64, :],
                    xw[64 * g_:64 * g_ + 64,
                       HEAD + 512 * ch:HEAD + 512 * (ch + 1)].bitcast(F32),
                    ALU.add,
                )
                (nc.scalar if g_ == 1 else nc.sync).dma_start(
                    _OUT[0][s, :, s0 + 512 * g_:s0 + 512 * (g_ + 1)],
                    out_sb[64 * g_:64 * g_ + 64, :])
            continue
        elif t in dve_slots:
            # split on the psum bank boundary: the successor scores matmul
            # into this tile's first bank can start after the first half
            if os.environ.get("K2_DSPLIT", "0") == "1":
                for hh in range(2):
                    nc.vector.tensor_scalar(
                        expT[:, CHUNK * t + 512 * hh:
                             CHUNK * t + 512 * (hh + 1)].bitcast(I16),
                        cur_sc[:, 512 * hh:512 * (hh + 1)],
                        BOFFC, 0.0, ALU.add, ALU.max,
                    )
            else:
                nc.vector.tensor_scalar(
                    expT[:, CHUNK * t:CHUNK * (t + 1)].bitcast(I16),
                    cur_sc[:], BOFFC, 0.0, ALU.add, ALU.max,
                )
        else:
            nc.scalar.activation(
                expT[:, CHUNK * t:CHUNK * (t + 1)], cur_sc[:], AF.Exp,
                scale=RA,
            )

        # ---- body 0 only: units must precede the scores that read their
        # phiT (in-order PE would deadlock otherwise); elsewhere units are
        # emitted at slot end so they never delay the scores chain
        if body_idx == 0 and t in (1, 2, 3) and units:
            units.pop(0)()

        # ---- scores for t+1 (or next body's tile 0)
        if t < NT - 1:
            cur_sc = pp_sc.tile([128, CHUNK], F32, tag="sc",
                                name=f"sc_{s}_{ch}_{t + 1}")
            sc_mm(cur_sc, t + 1, 0)
            sc_mm(cur_sc, t + 1, 1)
        elif nxt is not None:
            s2, ch2 = nxt
            ps_head = pp_sc.tile([128, CHUNK], F32, tag="sc",
                                 name=f"sc_{s2}_{ch2}_0")
            sc_mm(ps_head, 0, 0, s2, ch2)
            sc_mm(ps_head, 0, 1, s2, ch2)
            _HEAD[0] = ps_head

        # ---- prev-chunk attn
        if prev is not None and t in prev_sched:
            g, tts = prev_sched[t]
            if at_prev[g] is None:
                at_prev[g] = pp_at.tile([128, 260], F32, tag="at",
                                        name=f"at_{ps_}_{ch_}_{g}")
            attn_mm(at_prev[g], expT_, gwo_, g, tts, ("p", g))
            if tts[-1] == 7:
                norm_group("p", at_prev[g], oT_prev, g, ps_, ch_)
                if fine and g == 1:
                    _emit_finish(nc, pools, ps_, ch_, handles, oT_prev,
                                 f"tp_{ps_}_{ch_}", tp_prev)

        # ---- fine body: own attn
        if fine and t in _FINE_OWN:
            for g, tts in _FINE_OWN[t]:
                if at_own[g] is None:
                    if g == 0:
                        at_own[g] = pp_at.tile([128, 260], F32, tag="at",
                                               name=f"at_{s}_{ch}_{g}")
                    else:
                        # the 1-buf at-ring still holds own group 0; borrow
                        # a scores-ring tile (2 banks, first 260 cols used)
                        at_own[g] = pp_sc.tile([128, CHUNK], F32, tag="sc",
                                               name=f"at_{s}_{ch}_{g}")[:, 0:260]
                attn_mm(at_own[g], expT, gwo, g, tts, ("o", g))

        # ---- deferred phase-A units for the next sample: slot end, and
        # only at slots just after an at-group norm (3) or after both
        # groups are done (7) — the 1-buf at-ring frees exactly there
        if body_idx != 0 and t in (3, 7) and units:
            units.pop(0)()

    if fine:
        return None

    if prev is not None:
        _emit_finish(nc, pools, ps_, ch_, handles, oT_prev, f"tp_{ps_}_{ch_}",
                     tp_prev)
    return (s, ch, expT)


def build_nc():
    nc = bacc.Bacc("TRN2", target_bir_lowering=False, debug=False,
                   num_devices=NCORES)
    x_ext = nc.dram_tensor("x", [BLOC, 128, HEAD + 2048], F32R,
                           kind="ExternalInput").ap()
    ident_ext = nc.dram_tensor("ident", [128, 128], BF16,
                               kind="ExternalInput").ap()
    wb_ext = nc.dram_tensor("wb", [128, 640], BF16,
                            kind="ExternalInput").ap()
    out_ext = nc.dram_tensor("out", [BLOC, C, S], F32,
                             kind="ExternalOutput").ap()

    with tile.TileContext(nc) as tc:
        with (
            tc.tile_pool(name="wpool", bufs=1) as p_w,
            tc.tile_pool(name="samp", bufs=2) as p_samp,
            tc.tile_pool(name="chunk", bufs=2) as p_chunk,
            tc.tile_pool(name="ppsc", bufs=3, space="PSUM") as pp_sc,
            tc.tile_pool(name="ppat", bufs=1, space="PSUM") as pp_at,
            tc.tile_pool(name="ppmix", bufs=1, space="PSUM") as pp_mix,
        ):
            ident_sb = p_w.tile([128, 128], BF16, tag="ident")
            wb_sb = p_w.tile([128, 640], BF16, tag="wb")
            _IDENT[0] = ident_sb
            _WB[0] = wb_sb
            # dummy exp: hoist the ACT exp-table load off the critical
            # path (tiny memset so the table load starts immediately)
            dummy_sb = p_w.tile([1, 2], F32, tag="dummy_sb")
            nc.vector.memset(dummy_sb[:], 0)
            nc.scalar.activation(dummy_sb[0:1, 0:2], dummy_sb[0:1, 0:2],
                                 AF.Exp)
            _OUT[0] = out_ext
            pools = (pp_sc, pp_at, pp_mix, p_samp, p_chunk)
            handles = [None] * BLOC

            def mid_dmas():
                # wb is needed by unit 0 (~2.5us in); ident only by the
                # first finish (~12us); both ride after the first x block
                nc.sync.dma_start(wb_sb[:], wb_ext[:])
                nc.sync.dma_start(ident_sb[:], ident_ext[:])

            handles[0], units0 = _phase_a(nc, pools, 0, x_ext, mid=mid_dmas)
            units0[0]()
            prev = None
            pending = units0[1:]
            seq = [(s, ch) for s in range(BLOC) for ch in range(NCH)]
            seq = seq[:_STAGE]
            for i, (s, ch) in enumerate(seq):
                last = i == len(seq) - 1 and _STAGE >= 8
                nxt = None if i == len(seq) - 1 else seq[i + 1]
                take = 3 if i == 0 else 2
                prev = _emit_chunk(nc, pools, s, ch, handles, prev,
                                   body_idx=i, fine=last,
                                   units=pending[:take], nxt=nxt)
                pending = pending[take:]
                if ch == 0 and s + 1 < BLOC:
                    handles[s + 1], pending = _phase_a(nc, pools, s + 1,
                                                       x_ext)

    nc.compile()
    return nc


_NC_CACHE = None


def _get_nc():
    global _NC_CACHE
    if _NC_CACHE is None:
        _NC_CACHE = build_nc()
    return _NC_CACHE


def _pack_inputs(x, w_theta, w_phi, w_g, w_o, gamma_f):
    """Host-side packing: conv head + bf16 weight blocks + stacked x."""
    head = np.zeros((128, HEAD), dtype=np.float32)
    wct = np.zeros((64, 64), dtype=np.float32)
    wct[:, 0:8] = w_phi.T
    wct[:, 32:64] = w_g.T
    # conv lhsT: rows 0:64 = [wct|0], rows 64:128 = [0|wct]
    head[0:64, 0:64] = wct
    head[64:128, 64:128] = wct

    wb = np.zeros((128, 640), dtype=ml_dtypes.bfloat16)
    wthA = (A * w_theta).astype(ml_dtypes.bfloat16)      # [8, 64]
    wog = ((gamma_f * w_o).T).astype(ml_dtypes.bfloat16)  # [32, 64]
    # phiT lhsT variants T(2h+v) at cols 128*(2h+v):
    #   h: phi row block (0:8 even tile / 64:72 odd), v: k-half column
    for h in range(2):
        for v in range(2):
            c0 = 128 * (2 * h + v)
            wb[64 * h:64 * h + 8, c0 + 64 * v:c0 + 64 * v + 64] = wthA
    # wog variants: rows 32:64 (even) / 96:128 (odd)
    wb[32:64, 512:576] = wog
    wb[96:128, 576:640] = wog

    xr = x.reshape(B, 64, 4, 2, 512)
    x128 = np.ascontiguousarray(xr.transpose(0, 3, 1, 2, 4)).reshape(
        B, 128, 2048)
    xcat = np.zeros((B, 128, HEAD + 2048), dtype=np.float32)
    xcat[:, :, HEAD:] = x128
    xcat[0::BLOC, :, 0:HEAD] = head[None, :, :]
    ident = np.eye(128, dtype=ml_dtypes.bfloat16)
    return xcat, ident, wb


def kernel(x, w_theta, w_phi, w_g, w_o, gamma):
    x = np.ascontiguousarray(np.asarray(x, dtype=np.float32))
    w_theta = np.asarray(w_theta, dtype=np.float32)
    w_phi = np.asarray(w_phi, dtype=np.float32)
    w_g = np.asarray(w_g, dtype=np.float32)
    w_o = np.asarray(w_o, dtype=np.float32)
    gamma_f = float(np.asarray(gamma, dtype=np.float32))

    xcat, ident, wb = _pack_inputs(x, w_theta, w_phi, w_g, w_o, gamma_f)

    nc = _get_nc()
    in_maps = [
        {
            "x": np.ascontiguousarray(xcat[i * BLOC:(i + 1) * BLOC]),
            "ident": ident,
            "wb": wb,
        }
        for i in range(NCORES)
    ]
    res = run_bass_kernel_spmd(nc, in_maps, core_ids=list(range(NCORES)))
    out = np.concatenate([res.results[i]["out"] for i in range(NCORES)],
                         axis=0)
    return out.reshape(B, C, H, W).astype(np.float32)


if __name__ == "__main__":
    rng = np.random.default_rng(0)
    ins = {
        "x": rng.standard_normal((B, C, H, W), dtype=np.float32),
        "w_theta": (rng.standard_normal((8, 64)) / 8.0).astype(np.float32),
        "w_phi": (rng.standard_normal((8, 64)) / 8.0).astype(np.float32),
        "w_g": (rng.standard_normal((32, 64)) / np.sqrt(64)).astype(np.float32),
        "w_o": (rng.standard_normal((64, 32)) / np.sqrt(32)).astype(np.float32),
        "gamma": np.float32(0.7),
    }
    out = kernel(**ins)
    print("out", out.shape, out.dtype, np.abs(out).mean())
